# revision 1
# baseline (speedup 1.0000x reference)
"""Trainium2 Bass kernel for nn_MobileOptimizedSimpleClawMatrix.

Computation (per batch element b):
    vp  = x_v @ Wv.T + bv                     [L, D]
    lp  = x_l @ Wl.T + bl                     [L, D]
    sim = vp @ lp.T                           [L, L]
    attn = softmax(sim, axis=-1)
    av  = attn @ vp                           [L, D]
    al  = attn.T @ lp                         [L, D]
    out = concat([av, al], -1) @ Wo.T + bo    [L, D]

Sharding: batch B=8 across the 8 NeuronCores (data parallel, params
replicated).  Each core runs the full per-batch-element pipeline.

All GEMMs run as fp32r (TF32-class, ~1.6e-4 rounding) at full PE rate.
Softmax runs in fp32 on ACT/DVE.  attn (normalized) and avT are spilled
to DRAM scratch and re-read, which keeps the SBUF working set under the
192KB/partition budget.
"""

import os

os.environ.setdefault("JAX_PLATFORMS", "")

import numpy as np

B = 8
L = 2048  # tokens
D = 768  # feature dim
P = 128
NK = D // P  # 6 chunks over feature dim
NT = L // P  # 16 token blocks
NPAIR = NT // 2  # 8 i-block pairs (av accumulates over 256-wide i slices)
NJS = L // 256  # 8 j slices for the al/out phase

_CACHE = {}
SECTION_HOOK = None  # optional: called with (label, nc) at section starts


def _mark(label, nc):
    if SECTION_HOOK is not None:
        SECTION_HOOK(label, nc)


def _build_nc(n_reps: int = 1):
    from contextlib import ExitStack

    import concourse.bacc as bacc
    import concourse.mybir as mybir
    import concourse.tile as tile
    from concourse.masks import make_identity

    F32 = mybir.dt.float32
    F32R = mybir.dt.float32r
    Exp = mybir.ActivationFunctionType.Exp
    Identity = mybir.ActivationFunctionType.Identity
    X = mybir.AxisListType.X

    nc = bacc.Bacc(
        "TRN2", target_bir_lowering=False, debug=False, num_devices=B,
        num_swdge_queues=4,
    )

    # ---- DRAM I/O (per core; host pre-transposes x and W) ----
    xvT = nc.dram_tensor("xvT", [D, L], F32, kind="ExternalInput")
    xlT = nc.dram_tensor("xlT", [D, L], F32, kind="ExternalInput")
    wvT = nc.dram_tensor("wvT", [D, D], F32, kind="ExternalInput")  # Wv.T [d, e]
    wlT = nc.dram_tensor("wlT", [D, D], F32, kind="ExternalInput")  # Wl.T [d, e]
    woT = nc.dram_tensor("woT", [2 * D, D], F32, kind="ExternalInput")  # Wo.T [c, e]
    bv = nc.dram_tensor("bv", [D], F32, kind="ExternalInput")
    bl = nc.dram_tensor("bl", [D], F32, kind="ExternalInput")
    bo = nc.dram_tensor("bo", [D], F32, kind="ExternalInput")
    out = nc.dram_tensor("out", [L, D], F32, kind="ExternalOutput")

    # d-major views of DRAM scratch for chunked access
    xvT_v = xvT[:].rearrange("(k p) t -> p k t", p=P)
    xlT_v = xlT[:].rearrange("(k p) t -> p k t", p=P)
    wvT_v = wvT[:].rearrange("(k p) e -> p k e", p=P)
    wlT_v = wlT[:].rearrange("(k p) e -> p k e", p=P)
    woT_v = woT[:].rearrange("(k p) e -> p k e", p=P)

    with ExitStack() as ctx:
        tc = ctx.enter_context(tile.TileContext(nc))

        dram = ctx.enter_context(tc.tile_pool(name="dram", bufs=1, space="DRAM"))
        escratch = dram.tile([L, L], F32R)  # normalized attn
        avT_dram = dram.tile([D, L], F32R)  # aligned_vision^T
        escr_v = escratch[:].rearrange("(c p) j -> p c j", p=P)
        avT_v = avT_dram[:].rearrange("(m p) t -> p m t", p=P)

        # ---- persistent pools ----
        const = ctx.enter_context(tc.tile_pool(name="const", bufs=1))
        vp_pool = ctx.enter_context(tc.tile_pool(name="vp", bufs=1))
        lpT_pool = ctx.enter_context(tc.tile_pool(name="lpT", bufs=1))

        ident_f = const.tile([P, P], F32)
        make_identity(nc, ident_f[:])
        ident = const.tile([P, P], F32R)
        nc.vector.tensor_copy(ident[:], ident_f[:])
        ones_f = const.tile([1, P], F32)
        nc.gpsimd.memset(ones_f[:], 1.0)
        ones1 = const.tile([1, P], F32R)
        nc.vector.tensor_copy(ones1[:], ones_f[:])

        for _rep in range(n_reps):
            # persistent arrays
            vp_t = vp_pool.tile([P, NT, D], F32R, tag="vpslot")  # vp[i, d] token-major
            lpT_t = lpT_pool.tile([P, NK, L], F32R, tag="lpTslot")  # lp^T [d, t]

            # ================= Phase A: projections =================
            with ExitStack() as actx:
                wpool = actx.enter_context(tc.tile_pool(name="wts", bufs=1))
                xv_pool = actx.enter_context(tc.tile_pool(name="xv", bufs=3))
                xl_pool = actx.enter_context(tc.tile_pool(name="xl", bufs=2))
                stg_pool = actx.enter_context(tc.tile_pool(name="stg", bufs=2))
                xls_pool = actx.enter_context(tc.tile_pool(name="xls", bufs=1))
                pa_vp = actx.enter_context(tc.tile_pool(name="pa_vp", bufs=2, space="PSUM"))
                pa_lp = actx.enter_context(tc.tile_pool(name="pa_lp", bufs=2, space="PSUM"))

                # first x block + WvT chunks first so the PE starts ASAP;
                # WlT loads are emitted after the first vp block (hidden)
                xvb0 = xv_pool.tile([P, NK, P], F32R, tag="xvb")
                xstg0 = xv_pool.tile([P, NK, P], F32, tag="xstg", name="xstg0")
                nc.sync.dma_start(xstg0[:], xvT_v[:, :, 0:P])
                nc.vector.tensor_copy(xvb0[:], xstg0[:])
                w12 = wpool.tile([P, 2 * NK, D], F32R, tag="w12")
                for k in range(NK):
                    wstg = stg_pool.tile([P, D], F32, tag="wstg", name=f"wv{k}")
                    nc.sync.dma_start(wstg[:], wvT_v[:, k, :])
                    nc.vector.tensor_copy(w12[:, k, :], wstg[:])
                bvr = const.tile([1, D], F32R, tag="bvr", name="bvr")
                nc.gpsimd.dma_start(bvr[:], bv[:].unsqueeze(0))
                bor = const.tile([1, D], F32R, tag="bor", name="bor")
                nc.gpsimd.dma_start(bor[:], bo[:].unsqueeze(0))
                bl_col = const.tile([P, NK], F32, tag="bl_col", name="bl_col")
                nc.sync.dma_start(bl_col[:], bl[:].rearrange("(o p) -> p o", p=P))

                for g in range(4):  # groups of 4 token blocks
                    for tb in range(4 * g, 4 * g + 4):
                        if tb == 0:
                            xvb = xvb0
                        else:
                            xvb = xv_pool.tile([P, NK, P], F32R, tag="xvb")
                            xstg = xv_pool.tile(
                                [P, NK, P], F32, tag="xstg", name=f"xstg{tb}"
                            )
                            nc.sync.dma_start(
                                xstg[:], xvT_v[:, :, tb * P : (tb + 1) * P]
                            )
                            nc.vector.tensor_copy(xvb[:], xstg[:])
                        vps = pa_vp.tile([P, D], F32, tag="vps")
                        for k in range(NK):
                            nc.tensor.matmul(
                                vps[:, 0:512], xvb[:, k, :], w12[:, k, 0:512],
                                start=(k == 0), stop=False,
                            )
                            nc.tensor.matmul(
                                vps[:, 512:768], xvb[:, k, :], w12[:, k, 512:768],
                                start=(k == 0), stop=False,
                            )
                        nc.tensor.matmul(
                            vps[:, 0:512], ones1[:], bvr[:, 0:512], start=False, stop=True
                        )
                        nc.tensor.matmul(
                            vps[:, 512:768], ones1[:], bvr[:, 512:768],
                            start=False, stop=True,
                        )
                        nc.vector.tensor_copy(vp_t[:, tb, :], vps[:])
                        # spread WlT chunk loads across the first blocks
                        for k in {0: [0], 1: [1, 4], 2: [2, 5], 3: [3]}.get(tb, []):
                            wstg = stg_pool.tile(
                                [P, D], F32, tag="wstg", name=f"wl{k}"
                            )
                            nc.sync.dma_start(wstg[:], wlT_v[:, k, :])
                            nc.vector.tensor_copy(w12[:, NK + k, :], wstg[:])

                    # lpT for this 512-token slice
                    xlg = xl_pool.tile([P, NK, 512], F32R, tag="xlg")
                    xlstg = xls_pool.tile(
                        [P, NK, 512], F32, tag="xlstg", name=f"xlstg{g}"
                    )
                    nc.sync.dma_start(
                        xlstg[:], xlT_v[:, :, g * 512 : (g + 1) * 512]
                    )
                    nc.vector.tensor_copy(xlg[:], xlstg[:])
                    for me in range(NK):
                        lps = pa_lp.tile([P, 512], F32, tag="lps")
                        for k in range(NK):
                            nc.tensor.matmul(
                                lps[:], w12[:, NK + k, me * P : (me + 1) * P],
                                xlg[:, k, :],
                                start=(k == 0), stop=(k == NK - 1),
                            )
                        nc.scalar.activation(
                            lpT_t[:, me, g * 512 : (g + 1) * 512], lps[:],
                            Identity, bias=bl_col[:, me : me + 1], scale=1.0,
                        )

            # ================= Phase B: attention + av =================
            # Deep software pipeline, all deferred so the PE never waits on
            # softmax:
            #   block k emits:  vpT-tr(k), sim(k) (+per-slice evac/reduce),
            #                   one av chunk of pair (k-3)//2, attnT-tr(k-1),
            #                   softmax(k)+spill
            # attn^T stays in SBUF; av accumulates straight from it.
            with ExitStack() as bctx:
                vpT_pool = bctx.enter_context(tc.tile_pool(name="vpT", bufs=2))
                simsb_pool = bctx.enter_context(tc.tile_pool(name="simsb", bufs=2))
                attn_pool = bctx.enter_context(tc.tile_pool(name="attn", bufs=3))
                attnT_pool = bctx.enter_context(tc.tile_pool(name="attnT", bufs=2))
                avev_pool = bctx.enter_context(tc.tile_pool(name="avev", bufs=2))
                stat_pool = bctx.enter_context(tc.tile_pool(name="stat", bufs=4))
                pb_sim = bctx.enter_context(
                    tc.tile_pool(name="pb_sim", bufs=1, space="PSUM")
                )
                pb_tr = bctx.enter_context(
                    tc.tile_pool(name="pb_tr", bufs=2, space="PSUM")
                )
                pb_av = bctx.enter_context(
                    tc.tile_pool(name="pb_av", bufs=2, space="PSUM")
                )

                attn_tiles = {}
                pair_tiles = {}
                avev_tiles = {}
                pending = []  # (pair, md_lo, md_hi)

                def emit_av_chunk():
                    pair_, md_lo, md_hi = pending.pop(0)
                    attnT_ = pair_tiles[pair_]
                    if pair_ not in avev_tiles:
                        avev_tiles[pair_] = avev_pool.tile(
                            [P, NK, 256], F32R, tag="avev", name=f"avev_{pair_}"
                        )
                    avev_ = avev_tiles[pair_]
                    for md in range(md_lo, md_hi):
                        avp = pb_av.tile([P, 256], F32, tag="avp")
                        for jc in range(NT):
                            nc.tensor.matmul(
                                avp[:],
                                vp_t[:, jc, md * P : (md + 1) * P],
                                attnT_[:, jc, :],
                                start=(jc == 0), stop=(jc == NT - 1),
                            )
                        nc.scalar.copy(avev_[:, md, :], avp[:])
                    if md_hi == NK:
                        nc.sync.dma_start(
                            avT_v[:, :, pair_ * 256 : (pair_ + 1) * 256], avev_[:]
                        )
                        del pair_tiles[pair_], avev_tiles[pair_]

                def emit_attnT_tr(k):
                    pair_, half_ = divmod(k, 2)
                    if half_ == 0:
                        pair_tiles[pair_] = attnT_pool.tile(
                            [P, NT, 256], F32R, tag="attnTp", name=f"attnTp_{pair_}"
                        )
                    tile_ = pair_tiles[pair_]
                    a = attn_tiles.pop(k)
                    for jc2 in range(0, NT, 2):
                        ptr3 = pb_tr.tile([P, 3, P], F32R, tag="ptr3", name="ptrb")
                        ptr = ptr3[:, 0:2, :]
                        nc.tensor.transpose(
                            ptr[:, 0, :], a[:, jc2 * P : (jc2 + 1) * P], ident[:]
                        )
                        nc.tensor.transpose(
                            ptr[:, 1, :],
                            a[:, (jc2 + 1) * P : (jc2 + 2) * P],
                            ident[:],
                        )
                        nc.scalar.copy(
                            tile_[:, jc2 : jc2 + 2, half_ * P : (half_ + 1) * P],
                            ptr[:],
                        )
                    if half_ == 1:
                        pending.append((pair_, 0, 3))
                        pending.append((pair_, 3, NK))

                for iblk in range(NT):
                    # vpT slices for this block
                    vpTb = vpT_pool.tile([P, NK, P], F32R, tag="vpTb")
                    for k3 in range(0, NK, 3):
                        ptr = pb_tr.tile([P, 3, P], F32R, tag="ptr3")
                        for j in range(3):
                            nc.tensor.transpose(
                                ptr[:, j, :],
                                vp_t[:, iblk, (k3 + j) * P : (k3 + j + 1) * P],
                                ident[:],
                            )
                        nc.vector.tensor_copy(vpTb[:, k3 : k3 + 3, :], ptr[:])
                    # sim row block [128, 2048]; each 512-slice is evacuated
                    # to SBUF and max-reduced as soon as it completes.
                    sim = pb_sim.tile([P, L], F32, tag="sim")
                    simsb = simsb_pool.tile([P, L], F32, tag="simsb")
                    for ns in range(4):
                        for k in range(NK):
                            nc.tensor.matmul(
                                sim[:, ns * 512 : (ns + 1) * 512],
                                vpTb[:, k, :],
                                lpT_t[:, k, ns * 512 : (ns + 1) * 512],
                                start=(k == 0), stop=(k == NK - 1),
                            )
                        sl = slice(ns * 512, (ns + 1) * 512)
                        if ns % 2 == 0:
                            nc.scalar.copy(simsb[:, sl], sim[:, sl])
                        else:
                            nc.vector.tensor_copy(simsb[:, sl], sim[:, sl])
                    # fill with av work of an older, completed pair
                    if pending:
                        emit_av_chunk()
                    # deferred attn transposes of the previous block
                    if iblk >= 1:
                        emit_attnT_tr(iblk - 1)
                    # softmax (rows)
                    negm = stat_pool.tile([P, 1], F32, tag="negm")
                    nc.vector.reduce_max(negm[:], simsb[:], axis=X, negate=True)
                    attn = attn_pool.tile(
                        [P, L], F32R, tag="attn", name=f"attn_{iblk}"
                    )
                    attn_tiles[iblk] = attn
                    z = stat_pool.tile([P, 1], F32, tag="z")
                    nc.scalar.activation(
                        attn[:], simsb[:], Exp, bias=negm[:], scale=1.0,
                        accum_out=z[:],
                    )
                    rz = stat_pool.tile([P, 1], F32, tag="rz")
                    nc.vector.reciprocal(rz[:], z[:])
                    nc.vector.tensor_scalar_mul(
                        attn[:, 0:1024], attn[:, 0:1024], rz[:]
                    )
                    nc.vector.tensor_scalar_mul(
                        attn[:, 1024:2048], attn[:, 1024:2048], rz[:]
                    )
                    # spill normalized attn row block
                    nc.sync.dma_start(
                        escratch[iblk * P : (iblk + 1) * P, :], attn[:]
                    )
                emit_attnT_tr(NT - 1)
                while pending:
                    emit_av_chunk()

            # ================= Phase C/D: al + output projection =================
            with ExitStack() as cctx:
                lp_t = vp_pool.tile([P, NT, D], F32R, tag="vpslot")  # reuse vp slot
                col_pool = cctx.enter_context(tc.tile_pool(name="col", bufs=2))
                alT_pool = cctx.enter_context(tc.tile_pool(name="alT", bufs=2))
                avtt_pool = cctx.enter_context(tc.tile_pool(name="avtt", bufs=2))
                outsb_pool = cctx.enter_context(tc.tile_pool(name="outsb", bufs=2))
                pc_tr = cctx.enter_context(tc.tile_pool(name="pc_tr", bufs=2, space="PSUM"))
                pc_al = cctx.enter_context(tc.tile_pool(name="pc_al", bufs=2, space="PSUM"))
                pd_out = cctx.enter_context(
                    tc.tile_pool(name="pd_out", bufs=2, space="PSUM")
                )

                # derive lp token-major from lpT (96 transposes)
                for ic in range(NT):
                    for k3 in range(0, NK, 3):
                        ptr = pc_tr.tile([P, 3, P], F32R, tag="ptr3")
                        for j in range(3):
                            nc.tensor.transpose(
                                ptr[:, j, :],
                                lpT_t[:, k3 + j, ic * P : (ic + 1) * P],
                                ident[:],
                            )
                        nc.scalar.copy(
                            lp_t[:, ic, k3 * P : (k3 + 3) * P], ptr[:]
                        )

                # woT weights
                wopool = cctx.enter_context(tc.tile_pool(name="wot", bufs=1))
                woT_t = wopool.tile([P, 2 * NK, D], F32R, tag="woT")
                for k in range(2 * NK):
                    nc.gpsimd.dma_start(woT_t[:, k, :], woT_v[:, k, :])

                for js in range(NJS):
                    colt = col_pool.tile([P, NT, 256], F32R, tag="colt")
                    nc.sync.dma_start(colt[:], escr_v[:, :, js * 256 : (js + 1) * 256])
                    alT = alT_pool.tile([P, NK, 256], F32R, tag="alT")
                    for md in range(NK):
                        alp = pc_al.tile([P, 256], F32, tag="alp")
                        for ic in range(NT):
                            nc.tensor.matmul(
                                alp[:],
                                lp_t[:, ic, md * P : (md + 1) * P],
                                colt[:, ic, :],
                                start=(ic == 0), stop=(ic == NT - 1),
                            )
                        nc.vector.tensor_copy(alT[:, md, :], alp[:])
                    # output projection for the 2 token tiles in this slice
                    for half in range(2):
                        tt = 2 * js + half
                        avtt = avtt_pool.tile([P, NK, P], F32R, tag="avtt")
                        nc.gpsimd.dma_start(avtt[:], avT_v[:, :, tt * P : (tt + 1) * P])
                        ops = pd_out.tile([P, D], F32, tag="ops")
                        for kc in range(2 * NK):
                            lhsT = (
                                avtt[:, kc, :]
                                if kc < NK
                                else alT[:, kc - NK, half * P : (half + 1) * P]
                            )
                            nc.tensor.matmul(
                                ops[:, 0:512], lhsT, woT_t[:, kc, 0:512],
                                start=(kc == 0), stop=False,
                            )
                            nc.tensor.matmul(
                                ops[:, 512:768], lhsT, woT_t[:, kc, 512:768],
                                start=(kc == 0), stop=False,
                            )
                        nc.tensor.matmul(
                            ops[:, 0:512], ones1[:], bor[:, 0:512], start=False, stop=True
                        )
                        nc.tensor.matmul(
                            ops[:, 512:768], ones1[:], bor[:, 512:768],
                            start=False, stop=True,
                        )
                        outsb = outsb_pool.tile([P, D], F32, tag="outsb")
                        nc.vector.tensor_copy(outsb[:], ops[:])
                        nc.sync.dma_start(out[tt * P : (tt + 1) * P, :], outsb[:])

    nc.compile()
    return nc


def _build_sharded(nc):
    """Cache a jitted sharded executable so repeat calls skip retracing."""
    import jax
    import concourse.mybir as mybir
    from jax.sharding import Mesh, PartitionSpec
    from jax.experimental.shard_map import shard_map
    from concourse.bass2jax import (
        _bass_exec_p,
        install_neuronx_cc_hook,
        partition_id_tensor,
    )

    install_neuronx_cc_hook()
    partition_name = nc.partition_id_tensor.name if nc.partition_id_tensor else None
    in_names, out_names, out_avals, zero_outs = [], [], [], []
    for alloc in nc.m.functions[0].allocations:
        if not isinstance(alloc, mybir.MemoryLocationSet):
            continue
        name = alloc.memorylocations[0].name
        if alloc.kind == "ExternalInput":
            if name != partition_name:
                in_names.append(name)
        elif alloc.kind == "ExternalOutput":
            shape = tuple(alloc.tensor_shape)
            dtype = mybir.dt.np(alloc.dtype)
            out_names.append(name)
            out_avals.append(jax.core.ShapedArray(shape, dtype))
            zero_outs.append(np.zeros(shape, dtype))
    n_params = len(in_names)
    n_outs = len(out_avals)
    all_in_names = list(in_names) + list(out_names)
    if partition_name is not None:
        all_in_names.append(partition_name)
    donate = tuple(range(n_params, n_params + n_outs))

    def _body(*args):
        operands = list(args)
        if partition_name is not None:
            operands.append(partition_id_tensor())
        return tuple(
            _bass_exec_p.bind(
                *operands,
                out_avals=tuple(out_avals),
                in_names=tuple(all_in_names),
                out_names=tuple(out_names),
                lowering_input_output_aliases=(),
                sim_require_finite=True,
                sim_require_nnan=True,
                nc=nc,
            )
        )

    devices = jax.devices()[:B]
    mesh = Mesh(np.asarray(devices), ("core",))
    sharding = jax.sharding.NamedSharding(mesh, PartitionSpec("core"))
    sharded = jax.jit(
        shard_map(
            _body,
            mesh=mesh,
            in_specs=(PartitionSpec("core"),) * (n_params + n_outs),
            out_specs=(PartitionSpec("core"),) * n_outs,
            check_rep=False,
        ),
        donate_argnums=donate,
        keep_unused=True,
    )

    import jax.numpy as jnp

    zero_shapes = tuple((B * z.shape[0], *z.shape[1:]) for z in zero_outs)
    zero_dtypes = tuple(z.dtype for z in zero_outs)

    @jax.jit
    def _make_zeros():
        return tuple(
            jnp.zeros(s, d) for s, d in zip(zero_shapes, zero_dtypes)
        )

    def device_zeros():
        return jax.device_put(_make_zeros(), [sharding] * len(zero_shapes))

    return {
        "sharded": sharded,
        "in_names": in_names,
        "out_names": out_names,
        "zero_outs": zero_outs,
        "out_avals": out_avals,
        "sharding": sharding,
        "device_zeros": device_zeros,
    }


def kernel(
    vision_features, language_features, Wv, bv, Wl, bl, Wo, bo
) -> np.ndarray:
    from concourse.bass_utils import run_bass_kernel_spmd

    nc = _CACHE.get("nc")
    if nc is None:
        nc = _build_nc()
        _CACHE["nc"] = nc

    wvT = np.ascontiguousarray(np.asarray(Wv, dtype=np.float32).T)
    wlT = np.ascontiguousarray(np.asarray(Wl, dtype=np.float32).T)
    woT = np.ascontiguousarray(np.asarray(Wo, dtype=np.float32).T)
    bv = np.asarray(bv, dtype=np.float32)
    bl = np.asarray(bl, dtype=np.float32)
    bo = np.asarray(bo, dtype=np.float32)
    vision_features = np.asarray(vision_features, dtype=np.float32)
    language_features = np.asarray(language_features, dtype=np.float32)

    in_maps = []
    for b in range(B):
        in_maps.append(
            {
                "xvT": np.ascontiguousarray(vision_features[b].T),
                "xlT": np.ascontiguousarray(language_features[b].T),
                "wvT": wvT,
                "wlT": wlT,
                "woT": woT,
                "bv": bv,
                "bl": bl,
                "bo": bo,
            }
        )

    try:
        ex = _CACHE.get("ex")
        if ex is None:
            ex = _build_sharded(nc)
            _CACHE["ex"] = ex
        concat_in = [
            np.concatenate([m[n] for m in in_maps], axis=0)
            for n in ex["in_names"]
        ]
        out_arrs = ex["sharded"](*concat_in, *ex["device_zeros"]())
        i = ex["out_names"].index("out")
        full = np.asarray(out_arrs[i]).reshape(B, *ex["out_avals"][i].shape)
        return full.astype(np.float32)
    except Exception:
        res = run_bass_kernel_spmd(nc, in_maps, list(range(B)))
        return np.stack([res.results[b]["out"] for b in range(B)]).astype(np.float32)

